# revision 23
# baseline (speedup 1.0000x reference)
"""RIENet loss kernel (keypoint/KNN MSE + global-align Huber-min loss) on 8 trn2 cores.

Sharding: core ci -> (b = ci // 4, n-chunk j = ci % 4).  Each core holds the full
tgt[b] (M=8192 points) and a 2048-column chunk of src_transformed[b] (N axis).
  loss_1 (min over M per src point): complete locally per core.
  loss_2 (min over N per tgt point): per-core partial min over its chunk;
          host min-reduces the 4 chunks per batch element.

Design (three-engine pipeline, host-side operand prep):
  Host builds bf16-split operand matrices tA [24, M] / sA [24, CHUNK] so that a
  single K=24 bf16 matmul produces the FULL squared-distance matrix
    P[m, n] = ||t_m||^2 + ||s_n||^2 - 2 t_m.s_n
  in fp32 PSUM (3-way bf16 splits of t, s and both norms; the 6 dominant
  cross products; abs err ~1e-5).  No device-side splits/transposes.

  Per 128-row m-tile (64 tiles), fully pipelined across three engines:
    PE:   4 x 512-col bf16 matmuls -> PSUM [128, 2048]        (~0.9 us)
    ACT:  copy PSUM -> SBUF bf16 q (the only idle engine)     (~2.0 us)
    DVE:  TT min   acc = min(acc, q)              (colmin, 2x mode, 1.2 us)
          TT min   rowq[mi] = min(q_lo, q_hi)     (rowmin fold,  2x, 0.7 us)
          every 16 tiles: halve those rowq slots 1024 -> 16 in place
  DVE min throughput is the bound: every P element passes the DVE twice at
  2 elem/cycle (bf16 2x_1P); ScalarE has no min, GPSIMD can't touch PSUM and
  the compiler rejects Pool tensor_tensor, so ~155 us of DVE is the floor.
  Tail: one 3D reduce for rowmin; partition-min of acc via 16 PE transposes
  into one PSUM tile + one 3D DVE reduce for colmin.
  Tiny keypoint/KNN MSE losses are computed on host in float64.
"""

import os
import numpy as np


def _ensure_path():
    try:
        import concourse  # noqa: F401
    except ImportError:
        import sys
        for p in ("/opt/trn_rl_repo", "/root/.axon_site/_ro/trn_rl_repo"):
            if os.path.isdir(p) and p not in sys.path:
                sys.path.insert(0, p)


_ensure_path()

import concourse.bass as bass  # noqa: E402
import concourse.bacc as bacc  # noqa: E402
import concourse.tile as tile  # noqa: E402
import concourse.mybir as mybir  # noqa: E402
from concourse.bass_utils import run_bass_kernel_spmd  # noqa: E402

F32 = mybir.dt.float32
BF16 = mybir.dt.bfloat16
AL = mybir.AluOpType

MARGIN = 0.1
B, KP, KNN, N, M = 2, 256, 32, 8192, 8192
NCORES = 8
NSHARDS = NCORES // B          # 4 n-chunks per batch element
CHUNK = N // NSHARDS           # 2048
NJ = CHUNK // 512              # 4 psum banks per m-tile
MI = M // 128                  # 64 m-tiles
K24 = 24
BIG = 3.0e38

_CACHE = {}
_BF16_NP = mybir.dt.np(BF16)


def _build():
    nc = bacc.Bacc("TRN2", target_bir_lowering=False, debug=False,
                   num_devices=NCORES)

    tA_d = nc.dram_tensor("tA", [K24, M], BF16, kind="ExternalInput")
    sA_d = nc.dram_tensor("sA", [K24, CHUNK], BF16, kind="ExternalInput")
    ident = nc.dram_tensor("ident", [128, 128], F32, kind="ExternalInput")

    colmin_o = nc.dram_tensor("colmin", [128, CHUNK // 128], F32,
                              kind="ExternalOutput")
    rowmin_o = nc.dram_tensor("rowmin", [128, MI], F32, kind="ExternalOutput")

    with tile.TileContext(nc) as tc:
        with (
            tc.tile_pool(name="const", bufs=1) as const,
            tc.tile_pool(name="qp", bufs=3) as qp,
        ):
            tA = const.tile([K24, M], BF16)
            sA = const.tile([K24, CHUNK], BF16)
            id_sb = const.tile([128, 128], F32)
            accf32 = const.tile([128, CHUNK], F32)
            accbf = const.tile([128, CHUNK], BF16)
            rowq = const.tile([128, MI, 1024], BF16)
            rowbuf = const.tile([128, MI], F32)
            colmin_sb = const.tile([128, CHUNK // 128], F32)

            nc.sync.dma_start(out=sA[:], in_=sA_d[:])
            nc.sync.dma_start(out=tA[:, 0:1024], in_=tA_d[:, 0:1024])
            nc.sync.dma_start(out=tA[:, 1024:M], in_=tA_d[:, 1024:M])
            nc.sync.dma_start(out=id_sb[:], in_=ident[:])
            nc.gpsimd.memset(accbf[:], BIG)

            # ---- main loop: P = nt + ns - 2 t.s per 128-row m-tile ----
            with tc.tile_pool(name="psum_main", bufs=2, space="PSUM") as pm:
                for mi in range(MI):
                    pt = pm.tile([128, CHUNK], F32, tag="pt")
                    for nj in range(NJ):
                        nc.tensor.matmul(
                            pt[:, nj * 512:(nj + 1) * 512],
                            lhsT=tA[:, mi * 128:(mi + 1) * 128],
                            rhs=sA[:, nj * 512:(nj + 1) * 512],
                            start=True, stop=True,
                        )
                    qbf = qp.tile([128, CHUNK], BF16, tag="qbf")
                    nc.scalar.copy(out=qbf[:], in_=pt[:])
                    # rowmin fold 2048 -> 1024 into this tile's rowq slot
                    nc.vector.tensor_tensor(
                        out=rowq[:, mi, :], in0=qbf[:, 0:1024],
                        in1=qbf[:, 1024:2048], op=AL.min)
                    # colmin partial: acc = min(acc, q)
                    nc.vector.tensor_tensor(
                        out=accbf[:], in0=qbf[:], in1=accbf[:], op=AL.min)
                    # interleaved batch rowmin: after each group of 16 tiles,
                    # halve those 16 rowq slots 1024 -> 16 in place
                    if mi % 16 == 15:
                        g = mi - 15
                        w = 1024
                        while w > 16:
                            h = w // 2
                            nc.vector.tensor_tensor(
                                out=rowq[:, g:g + 16, 0:h],
                                in0=rowq[:, g:g + 16, 0:h],
                                in1=rowq[:, g:g + 16, h:w], op=AL.min)
                            w = h

                nc.vector.tensor_reduce(
                    out=rowbuf[:], in_=rowq[:, :, 0:16],
                    axis=mybir.AxisListType.X, op=AL.min)
            nc.sync.dma_start(out=rowmin_o[:], in_=rowbuf[:])

            with tc.tile_pool(name="psum_fin", bufs=1, space="PSUM") as pf:
                # partition-axis min of accbf via PE transposes (fp32 path)
                nc.scalar.copy(out=accf32[:], in_=accbf[:])
                tp = pf.tile([128, CHUNK // 128, 128], F32)
                for blk in range(CHUNK // 128):
                    nc.tensor.transpose(tp[:, blk, :],
                                        accf32[:, blk * 128:(blk + 1) * 128],
                                        id_sb[:])
                nc.vector.tensor_reduce(
                    out=colmin_sb[:], in_=tp[:],
                    axis=mybir.AxisListType.X, op=AL.min)

            nc.sync.dma_start(out=colmin_o[:], in_=colmin_sb[:])

    nc.compile()
    return nc


def _get_nc():
    if "nc" not in _CACHE:
        _CACHE["nc"] = _build()
    return _CACHE["nc"]


def _split3(x):
    """3-way bf16 split of a float array (computed in float64/float32)."""
    x = np.asarray(x, dtype=np.float64)
    h = x.astype(_BF16_NP)
    r1 = x - h.astype(np.float64)
    m = r1.astype(_BF16_NP)
    r2 = r1 - m.astype(np.float64)
    l = r2.astype(_BF16_NP)
    return h, m, l


def _prepare_in_maps(src_transformed, tgt):
    f = np.float32
    st = np.asarray(src_transformed, dtype=f)
    tg = np.asarray(tgt, dtype=f)

    ident = np.eye(128, dtype=f)

    in_maps = []
    for ci in range(NCORES):
        b, j = divmod(ci, NSHARDS)
        t = tg[b]                                    # (3, M)
        s = st[b, :, j * CHUNK:(j + 1) * CHUNK]      # (3, CHUNK)

        th, tm, tl = _split3(t)
        sh, sm, sl = _split3(s)
        nt = np.sum(t.astype(np.float64) ** 2, axis=0)
        ns = np.sum(s.astype(np.float64) ** 2, axis=0)
        nth, ntm, ntl = _split3(nt)
        nsh, nsm, nsl = _split3(ns)

        def neg2(a):
            return (a.astype(f) * -2.0).astype(_BF16_NP)

        tA = np.zeros((K24, M), dtype=_BF16_NP)
        sA = np.zeros((K24, CHUNK), dtype=_BF16_NP)
        # cross products: (th,sh) (th,sm) (tm,sh) (tm,sm) (th,sl) (tl,sh)
        pairs = [(th, sh), (th, sm), (tm, sh), (tm, sm), (th, sl), (tl, sh)]
        r = 0
        for tp_, sp_ in pairs:
            for d in range(3):
                tA[r] = neg2(tp_[d])
                sA[r] = sp_[d]
                r += 1
        # ||s||^2 rows: ones x ns splits
        for part in (nsh, nsm, nsl):
            tA[r] = np.ones(M, dtype=_BF16_NP)
            sA[r] = part
            r += 1
        # ||t||^2 rows: nt splits x ones
        for part in (nth, ntm, ntl):
            tA[r] = part
            sA[r] = np.ones(CHUNK, dtype=_BF16_NP)
            r += 1
        assert r == K24

        in_maps.append({
            "tA": np.ascontiguousarray(tA),
            "sA": np.ascontiguousarray(sA),
            "ident": ident,
        })
    return in_maps


def _huber(x, c):
    return np.where(x < c, 0.5 * x * x, c * x - 0.5 * c * c)


def _postprocess(results):
    c = np.float64(MARGIN)
    loss1 = np.float64(0.0)
    loss2 = np.float64(0.0)
    for b in range(B):
        rowmins = []
        for j in range(NSHARDS):
            r = results[b * NSHARDS + j]
            colmin = np.asarray(r["colmin"], dtype=np.float64).T.ravel()
            loss1 += _huber(colmin, c).sum()
            rowmins.append(np.asarray(r["rowmin"], dtype=np.float64).T.ravel())
        rm = np.minimum.reduce(rowmins)
        loss2 += _huber(rm, c).sum()
    return loss1 + loss2


def run_device(in_maps, **kw):
    nc = _get_nc()
    return run_bass_kernel_spmd(nc, in_maps, list(range(NCORES)), **kw)


def _ncl_host(src_keypoints, tgt_keypoints, rotation_ab, translation_ab,
              src_keypoints_knn, tgt_keypoints_knn):
    f64 = np.float64
    skp = np.asarray(src_keypoints, dtype=f64)
    tkp = np.asarray(tgt_keypoints, dtype=f64)
    rot = np.asarray(rotation_ab, dtype=f64)
    tra = np.asarray(translation_ab, dtype=f64)
    sknn = np.asarray(src_keypoints_knn, dtype=f64)
    tknn = np.asarray(tgt_keypoints_knn, dtype=f64)

    transformed = np.einsum('bij,bjk->bik', rot, skp) + tra[:, :, None]
    kp_sq = (transformed - tkp) ** 2
    keypoints_loss = np.mean(np.sum(kp_sq, axis=(1, 2)))
    knn_sq = (sknn - tknn) ** 2
    knn_loss = np.mean(np.sum(knn_sq, axis=(1, 2)))
    return knn_loss + keypoints_loss


def kernel(src_keypoints, tgt_keypoints, rotation_ab, translation_ab,
           src_keypoints_knn, tgt_keypoints_knn, k, src_transformed, tgt,
           **_unused):
    in_maps = _prepare_in_maps(src_transformed, tgt)
    res = run_device(in_maps)
    gal = _postprocess(res.results)
    ncl = _ncl_host(src_keypoints, tgt_keypoints, rotation_ab, translation_ab,
                    src_keypoints_knn, tgt_keypoints_knn)
    return np.float32(ncl), np.float32(gal)


# revision 26
# speedup vs baseline: 1.0004x; 1.0004x over previous
"""RIENet loss kernel (keypoint/KNN MSE + global-align Huber-min loss) on 8 trn2 cores.

Sharding: core ci -> (b = ci // 4, n-chunk j = ci % 4).  Each core holds the full
tgt[b] (M=8192 points) and a 2048-column chunk of src_transformed[b] (N axis).
  loss_1 (min over M per src point): complete locally per core.
  loss_2 (min over N per tgt point): per-core partial min over its chunk;
          host min-reduces the 4 chunks per batch element.

Design (three-engine pipeline, host-side operand prep):
  Host builds bf16-split operand matrices tA [24, M] / sA [24, CHUNK] so that a
  single K=24 bf16 matmul produces the FULL squared-distance matrix
    P[m, n] = ||t_m||^2 + ||s_n||^2 - 2 t_m.s_n
  in fp32 PSUM (3-way bf16 splits of t, s and both norms; the 6 dominant
  cross products; abs err ~1e-5).  No device-side splits/transposes.

  Per 128-row m-tile (64 tiles), fully pipelined across three engines:
    PE:   4 x 512-col bf16 matmuls -> PSUM [128, 2048]        (~0.9 us)
    ACT:  copy PSUM -> SBUF bf16 q (the only idle engine)     (~2.0 us)
    DVE:  TT min   acc = min(acc, q)              (colmin, 2x mode, 1.2 us)
          TT min   rowq[mi] = min(q_lo, q_hi)     (rowmin fold,  2x, 0.7 us)
          every 16 tiles: halve those rowq slots 1024 -> 16 in place
  DVE min throughput is the bound: every P element passes the DVE twice at
  2 elem/cycle (bf16 2x_1P); ScalarE has no min, GPSIMD can't touch PSUM and
  the compiler rejects Pool tensor_tensor, so ~155 us of DVE is the floor.
  Tail: one 3D reduce for rowmin; partition-min of acc via 16 PE transposes
  into one PSUM tile + one 3D DVE reduce for colmin.
  Tiny keypoint/KNN MSE losses are computed on host in float64.
"""

import os
import numpy as np


def _ensure_path():
    try:
        import concourse  # noqa: F401
    except ImportError:
        import sys
        for p in ("/opt/trn_rl_repo", "/root/.axon_site/_ro/trn_rl_repo"):
            if os.path.isdir(p) and p not in sys.path:
                sys.path.insert(0, p)


_ensure_path()

import concourse.bass as bass  # noqa: E402
import concourse.bacc as bacc  # noqa: E402
import concourse.tile as tile  # noqa: E402
import concourse.mybir as mybir  # noqa: E402
from concourse.bass_utils import run_bass_kernel_spmd  # noqa: E402

F32 = mybir.dt.float32
BF16 = mybir.dt.bfloat16
AL = mybir.AluOpType

MARGIN = 0.1
B, KP, KNN, N, M = 2, 256, 32, 8192, 8192
NCORES = 8
NSHARDS = NCORES // B          # 4 n-chunks per batch element
CHUNK = N // NSHARDS           # 2048
NJ = CHUNK // 512              # 4 psum banks per m-tile
MI = M // 128                  # 64 m-tiles
K24 = 24
BIG = 3.0e38

_CACHE = {}
_BF16_NP = mybir.dt.np(BF16)


def _build():
    nc = bacc.Bacc("TRN2", target_bir_lowering=False, debug=False,
                   num_devices=NCORES)

    tA_d = nc.dram_tensor("tA", [K24, M], BF16, kind="ExternalInput")
    sA_d = nc.dram_tensor("sA", [K24, CHUNK], BF16, kind="ExternalInput")
    ident = nc.dram_tensor("ident", [128, 128], F32, kind="ExternalInput")

    colmin_o = nc.dram_tensor("colmin", [128, CHUNK // 128], F32,
                              kind="ExternalOutput")
    rowmin_o = nc.dram_tensor("rowmin", [128, MI], F32, kind="ExternalOutput")

    with tile.TileContext(nc) as tc:
        with tc.tile_pool(name="const", bufs=1) as const:
            tA = const.tile([K24, M], BF16)
            sA = const.tile([K24, CHUNK], BF16)
            id_sb = const.tile([128, 128], F32)
            accf32 = const.tile([128, CHUNK], F32)
            accbf = const.tile([128, CHUNK], BF16)
            qring = const.tile([128, 4, CHUNK], BF16)
            rowq = const.tile([128, MI, 1024], BF16)
            rowbuf = const.tile([128, MI], F32)
            colmin_sb = const.tile([128, CHUNK // 128], F32)

            nc.sync.dma_start(out=sA[:], in_=sA_d[:])
            nc.sync.dma_start(out=tA[:, 0:1024], in_=tA_d[:, 0:1024])
            nc.sync.dma_start(out=tA[:, 1024:M], in_=tA_d[:, 1024:M])
            nc.sync.dma_start(out=id_sb[:], in_=ident[:])
            nc.gpsimd.memset(accbf[:], BIG)

            # ---- main loop: P = nt + ns - 2 t.s per 128-row m-tile ----
            with tc.tile_pool(name="psum_main", bufs=2, space="PSUM") as pm:
                for mi in range(MI):
                    pt = pm.tile([128, CHUNK], F32, tag="pt")
                    for nj in range(NJ):
                        nc.tensor.matmul(
                            pt[:, nj * 512:(nj + 1) * 512],
                            lhsT=tA[:, mi * 128:(mi + 1) * 128],
                            rhs=sA[:, nj * 512:(nj + 1) * 512],
                            start=True, stop=True,
                        )
                    s = mi % 4
                    qbf = qring[:, s, :]
                    nc.scalar.copy(out=qbf, in_=pt[:])
                    # colmin partial: acc = min(acc, q)
                    nc.vector.tensor_tensor(
                        out=accbf[:], in0=qbf, in1=accbf[:], op=AL.min)
                    # rowmin fold 2048 -> 1024, two tiles per op (ring slots
                    # mi-1, mi are adjacent since pairs align to even mi)
                    if mi % 2 == 1:
                        nc.vector.tensor_tensor(
                            out=rowq[:, mi - 1:mi + 1, :],
                            in0=qring[:, s - 1:s + 1, 0:1024],
                            in1=qring[:, s - 1:s + 1, 1024:2048], op=AL.min)
                    # interleaved batch rowmin: after each group of 16 tiles,
                    # halve those 16 rowq slots 1024 -> 16 in place
                    if mi % 16 == 15:
                        g = mi - 15
                        w = 1024
                        while w > 16:
                            h = w // 2
                            nc.vector.tensor_tensor(
                                out=rowq[:, g:g + 16, 0:h],
                                in0=rowq[:, g:g + 16, 0:h],
                                in1=rowq[:, g:g + 16, h:w], op=AL.min)
                            w = h

                nc.vector.tensor_reduce(
                    out=rowbuf[:], in_=rowq[:, :, 0:16],
                    axis=mybir.AxisListType.X, op=AL.min)
            nc.sync.dma_start(out=rowmin_o[:], in_=rowbuf[:])

            with tc.tile_pool(name="psum_fin", bufs=1, space="PSUM") as pf:
                # partition-axis min of accbf via PE transposes (fp32 path)
                nc.scalar.copy(out=accf32[:], in_=accbf[:])
                tp = pf.tile([128, CHUNK // 128, 128], F32)
                for blk in range(CHUNK // 128):
                    nc.tensor.transpose(tp[:, blk, :],
                                        accf32[:, blk * 128:(blk + 1) * 128],
                                        id_sb[:])
                nc.vector.tensor_reduce(
                    out=colmin_sb[:], in_=tp[:],
                    axis=mybir.AxisListType.X, op=AL.min)

            nc.sync.dma_start(out=colmin_o[:], in_=colmin_sb[:])

    nc.compile()
    return nc


def _get_nc():
    if "nc" not in _CACHE:
        _CACHE["nc"] = _build()
    return _CACHE["nc"]


def _split3(x):
    """3-way bf16 split of a float array (computed in float64/float32)."""
    x = np.asarray(x, dtype=np.float64)
    h = x.astype(_BF16_NP)
    r1 = x - h.astype(np.float64)
    m = r1.astype(_BF16_NP)
    r2 = r1 - m.astype(np.float64)
    l = r2.astype(_BF16_NP)
    return h, m, l


def _prepare_in_maps(src_transformed, tgt):
    f = np.float32
    st = np.asarray(src_transformed, dtype=f)
    tg = np.asarray(tgt, dtype=f)

    ident = np.eye(128, dtype=f)

    in_maps = []
    for ci in range(NCORES):
        b, j = divmod(ci, NSHARDS)
        t = tg[b]                                    # (3, M)
        s = st[b, :, j * CHUNK:(j + 1) * CHUNK]      # (3, CHUNK)

        th, tm, tl = _split3(t)
        sh, sm, sl = _split3(s)
        nt = np.sum(t.astype(np.float64) ** 2, axis=0)
        ns = np.sum(s.astype(np.float64) ** 2, axis=0)
        nth, ntm, ntl = _split3(nt)
        nsh, nsm, nsl = _split3(ns)

        def neg2(a):
            return (a.astype(f) * -2.0).astype(_BF16_NP)

        tA = np.zeros((K24, M), dtype=_BF16_NP)
        sA = np.zeros((K24, CHUNK), dtype=_BF16_NP)
        # cross products: (th,sh) (th,sm) (tm,sh) (tm,sm) (th,sl) (tl,sh)
        pairs = [(th, sh), (th, sm), (tm, sh), (tm, sm), (th, sl), (tl, sh)]
        r = 0
        for tp_, sp_ in pairs:
            for d in range(3):
                tA[r] = neg2(tp_[d])
                sA[r] = sp_[d]
                r += 1
        # ||s||^2 rows: ones x ns splits
        for part in (nsh, nsm, nsl):
            tA[r] = np.ones(M, dtype=_BF16_NP)
            sA[r] = part
            r += 1
        # ||t||^2 rows: nt splits x ones
        for part in (nth, ntm, ntl):
            tA[r] = part
            sA[r] = np.ones(CHUNK, dtype=_BF16_NP)
            r += 1
        assert r == K24

        in_maps.append({
            "tA": np.ascontiguousarray(tA),
            "sA": np.ascontiguousarray(sA),
            "ident": ident,
        })
    return in_maps


def _huber(x, c):
    return np.where(x < c, 0.5 * x * x, c * x - 0.5 * c * c)


def _postprocess(results):
    c = np.float64(MARGIN)
    loss1 = np.float64(0.0)
    loss2 = np.float64(0.0)
    for b in range(B):
        rowmins = []
        for j in range(NSHARDS):
            r = results[b * NSHARDS + j]
            colmin = np.asarray(r["colmin"], dtype=np.float64).T.ravel()
            loss1 += _huber(colmin, c).sum()
            rowmins.append(np.asarray(r["rowmin"], dtype=np.float64).T.ravel())
        rm = np.minimum.reduce(rowmins)
        loss2 += _huber(rm, c).sum()
    return loss1 + loss2


def run_device(in_maps, **kw):
    nc = _get_nc()
    return run_bass_kernel_spmd(nc, in_maps, list(range(NCORES)), **kw)


def _ncl_host(src_keypoints, tgt_keypoints, rotation_ab, translation_ab,
              src_keypoints_knn, tgt_keypoints_knn):
    f64 = np.float64
    skp = np.asarray(src_keypoints, dtype=f64)
    tkp = np.asarray(tgt_keypoints, dtype=f64)
    rot = np.asarray(rotation_ab, dtype=f64)
    tra = np.asarray(translation_ab, dtype=f64)
    sknn = np.asarray(src_keypoints_knn, dtype=f64)
    tknn = np.asarray(tgt_keypoints_knn, dtype=f64)

    transformed = np.einsum('bij,bjk->bik', rot, skp) + tra[:, :, None]
    kp_sq = (transformed - tkp) ** 2
    keypoints_loss = np.mean(np.sum(kp_sq, axis=(1, 2)))
    knn_sq = (sknn - tknn) ** 2
    knn_loss = np.mean(np.sum(knn_sq, axis=(1, 2)))
    return knn_loss + keypoints_loss


def kernel(src_keypoints, tgt_keypoints, rotation_ab, translation_ab,
           src_keypoints_knn, tgt_keypoints_knn, k, src_transformed, tgt,
           **_unused):
    in_maps = _prepare_in_maps(src_transformed, tgt)
    res = run_device(in_maps)
    gal = _postprocess(res.results)
    ncl = _ncl_host(src_keypoints, tgt_keypoints, rotation_ab, translation_ab,
                    src_keypoints_knn, tgt_keypoints_knn)
    return np.float32(ncl), np.float32(gal)
